# revision 21
# baseline (speedup 1.0000x reference)
"""DeepPheno model kernel for 8 TRN2 NeuronCores — collective-free design.

Computation (reference):
    h    = gelu(gos @ W1 + b1)                     (B, HID)     erf-gelu
    x    = concat([h, exp_x], 1)                   (B, HID+EXP)
    flat = sigmoid(x @ W2 + b2)                    (B, C)
    out  = max_i flat[b, j] * M[i, j]              (B, C)

Since flat = sigmoid(..) > 0, the max-pool factorizes exactly:
    out[b, j] = flat[b, j] * max_i M[i, j]

Why no collectives: on this stack the first collective of an execution
pays a ~54us ncfw entry barrier plus ~30us+ of trigger->data latency, a
~90us serial chain that dominates the whole kernel (the previous sharded
design sat at ~95-110us because of it). Instead every core redundantly
computes matmul1 from the FULL W1, which is affordable because W1 is
carried in fp8e4m3 (host-cast): 15.7MB/core streams at ~354GB/s in ~44us,
fully overlapped with the matmul1 that consumes it.

Sharding: matmul1 fully replicated; W2 / b2 / hpo colmax / output are
split by class columns (core c owns classes [256c, 256(c+1))).

matmul1 runs "flipped" (h, not h.T): the tiny gos tile (128, 2, 64) is
the stationary operand (so the 15.7MB W1 stream pays no LDWEIGHTS) and
W1 streams as the moving operand in N=512 fp8 DoubleRow matmuls
(0.5 cycles/row, 256 contraction rows per instruction).

Precision: W1 is scaled by 64 on host before the e4m3 cast (raw W1
values ~N(0, 0.01) sit below e4m3's min normal 2^-6; scaling moves them
into the normal range; the gelu undoes it with scale=1/64). gos is cast
to e4m3 unscaled (values in [0,1)). Everything downstream is fp16/fp32:
h fp16, W2/exp fp16 (standard-mode matmul2, fp32 PSUM), sigmoid/colmax
multiply fp32, hpo matrix fp16 for the colmax. Measured rel_l2 ~6e-3
against the fp32 reference (gate 2e-2).

b1/b2 are folded into the matmuls: one zero-pad row of gos.T / x.T is
set to 1.0 and the matching W1 / W2 row carries the bias vector.
"""

import numpy as np
import ml_dtypes

import concourse.bacc as bacc
import concourse.mybir as mybir
import concourse.tile as tile
from concourse.bass_utils import run_bass_kernel_spmd
from concourse.masks import make_identity

# Problem shape (hardcoded per contract)
B = 64
IN = 10000
EXP = 53
HID = 1500
C = 2048

NCORES = 8
CD = C // NCORES        # 256 classes per core
KT1 = 80                # k tiles for matmul1: 80 * 128 = 10240 >= 10000 (even)
K1P = KT1 * 128
HIDP = HID              # no hid padding; small final block shortens the tail
BLK_W = [256, 256, 256, 256, 256, 156, 64]
BLK_OFF = [0, 256, 512, 768, 1024, 1280, 1436]
NBLK = len(BLK_W)
# k-tiles of x.T transposed after each block (tile fully written by then)
BLK_KTS = [[0, 1], [2, 3], [4, 5], [6, 7], [8, 9], [10], [11]]
KT2 = 13                # k tiles for matmul2: 11.72 h-tiles + exp/bias tile
K2P = KT2 * 128
W1SCALE = 64.0          # power of two; moves W1 into e4m3 normal range

F32 = mybir.dt.float32
F16 = mybir.dt.float16
F8 = mybir.dt.float8e4  # ml_dtypes.float8_e4m3

# k-tile DMA chunking per block (even sizes; final chunks small so the
# last-byte -> last-matmul catch-up is short)
W1_CHUNKS = [[40, 40]] * (NBLK - 1) + [[40, 22, 10, 6, 2]]


def _build_nc():
    nc = bacc.Bacc(
        "TRN2",
        target_bir_lowering=False,
        debug=False,
        enable_asserts=False,
        num_devices=NCORES,
    )

    # External I/O, all in SBUF-image layout (128, free)
    w1_d = nc.dram_tensor("w1_img", [128, KT1 * HIDP], F8, kind="ExternalInput")
    gos_d = nc.dram_tensor("gos_img", [128, KT1 * B], F8, kind="ExternalInput")
    w2_d = nc.dram_tensor("w2_img", [128, KT2 * CD], F16, kind="ExternalInput")
    exp_d = nc.dram_tensor("exp_img", [128, B], F16, kind="ExternalInput")
    mt_d = nc.dram_tensor("mt_img", [128, 2 * C], F16, kind="ExternalInput")
    out_d = nc.dram_tensor("out_img", [128, 2 * B], F32, kind="ExternalOutput")

    with tile.TileContext(nc) as tc:
        with (
            tc.tile_pool(name="big", bufs=1) as pp,
            tc.tile_pool(name="small", bufs=1) as sp,
            tc.tile_pool(name="ph", bufs=3, space="PSUM") as php,
            tc.tile_pool(name="pt", bufs=2, space="PSUM") as ptp,
            tc.tile_pool(name="pf", bufs=1, space="PSUM") as pfp,
        ):
            # identity for PE transposes (built on gpsimd, no DMA)
            ident = sp.tile([B, B], F16, tag="ident")
            make_identity(nc, ident[:, :])

            # --- small loads on the scalar HWDGE ring (Q10), off the W1 path.
            # gos rides Q10 too so Q1 is a pure W1 stream from the first byte.
            gos_sb = pp.tile([128, KT1 * B], F8, tag="gos")
            nc.scalar.dma_start(out=gos_sb[:, :], in_=gos_d[:, :])
            exp_sb = sp.tile([128, B], F16, tag="exp")
            nc.scalar.dma_start(out=exp_sb[:, :], in_=exp_d[:, :])
            w2_sb = sp.tile([128, KT2 * CD], F16, tag="w2")
            nc.scalar.dma_start(out=w2_sb[:, :], in_=w2_d[:, :])
            mt_sb = pp.tile([128, 2 * C], F16, tag="mt")
            cm_sb = sp.tile([128, 2], F32, tag="cm")
            for cb in range(2):
                sl = slice(cb * C, (cb + 1) * C)
                nc.scalar.dma_start(out=mt_sb[:, sl], in_=mt_d[:, sl])
                nc.vector.reduce_max(
                    cm_sb[:, cb : cb + 1], mt_sb[:, sl], axis=mybir.AxisListType.X
                )

            # --- W1 streamed block-major; matmul1 consumes chunk by chunk
            w1_sb = pp.tile([128, KT1 * HIDP], F8, tag="w1")
            h_sb = sp.tile([B, HIDP], F16, tag="h")
            xT_sb = sp.tile([128, KT2 * B], F16, tag="xT")
            # k-tile 11 of x.T covers h rows 1408..1499 only; zero the unused
            # partitions once so the (zero-W2-row) matmul2 products stay
            # finite (partition base must be 32-aligned; the transpose copy
            # later overwrites rows 64..91)
            nc.vector.memset(xT_sb[64:128, 11 * B : 12 * B], 0.0)
            psum_f = [
                pfp.tile([128, B], F32, tag=f"pf{cb}", name=f"pf{cb}")
                for cb in range(2)
            ]

            def mm2(cb, kt, start, stop):
                w2sl = w2_sb[:, kt * CD + cb * 128 : kt * CD + cb * 128 + 128]
                nc.tensor.matmul(
                    psum_f[cb][:, :],
                    lhsT=w2sl,
                    rhs=(exp_sb[:, :] if kt == KT2 - 1 else xT_sb[:, kt * B : (kt + 1) * B]),
                    start=start,
                    stop=stop,
                )

            # all W1 DMAs up front (one queue, in stream order)
            for nb in range(NBLK):
                base = BLK_OFF[nb] * KT1
                w = BLK_W[nb]
                ci = 0
                for ch in W1_CHUNKS[nb]:
                    sl = slice(base + ci * w, base + (ci + ch) * w)
                    nc.sync.dma_start(out=w1_sb[:, sl], in_=w1_d[:, sl])
                    ci += ch

            def mm1_block(nb):
                base = BLK_OFF[nb] * KT1
                w = BLK_W[nb]
                psh = php.tile([B, w], F32, tag="ph", name="ph")
                for t in range(KT1 // 2):
                    nc.tensor.matmul(
                        psh[:, :],
                        lhsT=gos_sb[:, 2 * t * B : (2 * t + 2) * B].rearrange(
                            "p (k b) -> p k b", k=2
                        ),
                        rhs=w1_sb[
                            :, base + 2 * t * w : base + (2 * t + 2) * w
                        ].rearrange("p (k f) -> p k f", k=2),
                        start=(t == 0),
                        stop=(t == KT1 // 2 - 1),
                        perf_mode=mybir.MatmulPerfMode.DoubleRow,
                    )
                return psh

            def block_tail(nb, psh):
                off, w = BLK_OFF[nb], BLK_W[nb]
                # gelu undoes the host-side W1 scaling; erf gelu
                nc.scalar.activation(
                    h_sb[:, off : off + w],
                    psh[:, :],
                    mybir.ActivationFunctionType.Gelu,
                    scale=1.0 / W1SCALE,
                )
                # transpose the h k-tiles completed by this block, feed matmul2
                for kt in BLK_KTS[nb]:
                    tw = min(128, HIDP - kt * 128)  # k-tile 11 is 92 rows
                    pt = ptp.tile([128, B], F16, tag="pt")
                    nc.tensor.transpose(
                        pt[0:tw, :], h_sb[:, kt * 128 : kt * 128 + tw], ident[:, :]
                    )
                    nc.vector.tensor_copy(
                        xT_sb[0:tw, kt * B : (kt + 1) * B], pt[0:tw, :]
                    )
                for cb in range(2):
                    for kt in BLK_KTS[nb]:
                        mm2(cb, kt, start=False, stop=(kt == 11))

            # PE issue order: block nb's gelu/transpose/mm2 tail is queued
            # AFTER block nb+1's matmul1 stream, so the in-order PE queue
            # never stalls on the ACT engine mid-stream.
            prev = mm1_block(0)
            for cb in range(2):
                # exp/bias k-tile opens the psum_f groups; operands arrive
                # early on Q10, long before the first h transposes.
                mm2(cb, KT2 - 1, start=True, stop=False)
            for nb in range(1, NBLK):
                cur = mm1_block(nb)
                block_tail(nb - 1, prev)
                prev = cur
            block_tail(NBLK - 1, prev)

            # sigmoid(pre) on ACT, then the colmax scale on DVE (keeps the two
            # stages on different engines); outputs split over both rings
            f_sb = sp.tile([128, 2 * B], F32, tag="f")
            o_sb = sp.tile([128, 2 * B], F32, tag="o")
            for cb in range(2):
                nc.scalar.activation(
                    f_sb[:, cb * B : (cb + 1) * B],
                    psum_f[cb][:, :],
                    mybir.ActivationFunctionType.Sigmoid,
                )
                nc.vector.tensor_scalar_mul(
                    o_sb[:, cb * B : (cb + 1) * B],
                    f_sb[:, cb * B : (cb + 1) * B],
                    cm_sb[:, cb : cb + 1],
                )
                (nc.sync if cb == 0 else nc.scalar).dma_start(
                    out=out_d[:, cb * B : (cb + 1) * B],
                    in_=o_sb[:, cb * B : (cb + 1) * B],
                )

    nc.compile()
    return nc


_NC_CACHE = None


def _get_nc():
    global _NC_CACHE
    if _NC_CACHE is None:
        _NC_CACHE = _build_nc()
    return _NC_CACHE


def _prep_inputs(gos, exp_x, W1, b1, W2, b2, hpo_matrix):
    f = np.float32
    gos = np.asarray(gos, f)
    exp_x = np.asarray(exp_x, f)
    W1 = np.asarray(W1, f)
    b1 = np.asarray(b1, f)
    W2 = np.asarray(W2, f)
    b2 = np.asarray(b2, f)
    M = np.asarray(hpo_matrix, f)
    f8 = ml_dtypes.float8_e4m3

    # W1 padded to K1P rows; bias row at K1P-1 pairs with the gos ones-row
    W1p = np.zeros((K1P, HIDP), f)
    W1p[:IN] = W1
    W1p[K1P - 1] = b1
    W1p8 = (W1p * W1SCALE).astype(f8)
    w1_img = np.concatenate(
        [
            W1p8[:, o : o + w]
            .reshape(KT1, 128, w)
            .transpose(1, 0, 2)
            .reshape(128, KT1 * w)
            for o, w in zip(BLK_OFF, BLK_W)
        ],
        axis=1,
    )
    w1_img = np.ascontiguousarray(w1_img)

    # gos.T padded to K1P rows with the ones-row last (b1 fold)
    gosT = np.zeros((K1P, B), f)
    gosT[:IN] = gos.T
    gosT[K1P - 1] = 1.0
    gos_img = np.ascontiguousarray(
        gosT.astype(f8).reshape(KT1, 128, B).transpose(1, 0, 2).reshape(128, KT1 * B)
    )

    # exp/bias k-tile of x.T: rows 0..52 exp.T, row 53 ones (b2 fold)
    exp_img = np.zeros((128, B), np.float16)
    exp_img[:EXP] = exp_x.T.astype(np.float16)
    exp_img[EXP] = 1.0

    # W2 rows remapped to x.T layout: h in rows 0..1499 (k-tiles 0..11, the
    # last one ragged), exp in k-tile 12 rows 1536..1588, b2 row at 1589
    W2p = np.zeros((K2P, C), f)
    W2p[:HID] = W2[:HID]
    W2p[12 * 128 : 12 * 128 + EXP] = W2[HID:]
    W2p[12 * 128 + EXP] = b2
    W2p16 = W2p.astype(np.float16)

    in_maps = []
    for c in range(NCORES):
        c0 = CD * c
        w2_img = np.ascontiguousarray(
            W2p16[:, c0 : c0 + CD]
            .reshape(KT2, 128, CD)
            .transpose(1, 0, 2)
            .reshape(128, KT2 * CD)
        )
        mt = M[:, c0 : c0 + CD].T.astype(np.float16)  # (256, 2048)
        mt_img = np.ascontiguousarray(np.concatenate([mt[:128], mt[128:]], axis=1))
        in_maps.append(
            {
                "w1_img": w1_img,
                "gos_img": gos_img,
                "w2_img": w2_img,
                "exp_img": exp_img,
                "mt_img": mt_img,
            }
        )
    return in_maps


def _assemble_output(results):
    cols = []
    for r in results:
        o = r["out_img"]  # (128, 2B): [p, cb*B + b] = out[b, c0 + cb*128 + p]
        chunk = o.reshape(128, 2, B).transpose(1, 0, 2).reshape(CD, B)
        cols.append(chunk.T)  # (B, CD)
    return np.ascontiguousarray(np.concatenate(cols, axis=1))


def kernel(gos, exp_x, W1, b1, W2, b2, hpo_matrix, **kw):
    nc = _get_nc()
    in_maps = _prep_inputs(gos, exp_x, W1, b1, W2, b2, hpo_matrix)
    res = run_bass_kernel_spmd(nc, in_maps, core_ids=list(range(NCORES)))
    return _assemble_output(res.results)
